# revision 54
# baseline (speedup 1.0000x reference)
"""LocallyConnected2d Trainium2 kernel (bf16 W-PAIR, contiguous weight image).

y[b,o,h,w] = sum_{i,ky,kx} x[b,i,h+ky-1,w+kx-1] * weight[i,o,h,w,ky,kx] + bias[o,h,w]

Shapes: x [64,64,32,32], weight [64,64,32,32,3,3], bias [64,32,32] -> y [64,64,32,32].

Strategy
--------
Spatial sharding over H_out: 8 cores x 4 output rows each (x slab with halo).
All compute in bf16 (fp32 matmuls are 4 cycles/row on TRN2 vs 1 for bf16;
rel-err budget 2e-2 >> bf16's ~0.3%), PSUM accumulation in fp32.

Output columns processed in pairs (A=2t, B=2t+1) sharing one PSUM tile
[128, 64] = [couts of A | couts of B] x batch. Per pair and kernel row ky,
TWO matmuls with K=128, M=128, N=64 cover all six (ky, kx) contributions:

  M1 @ x1[rc(h+ky, 2t)]   : rhs = [x(., 2t) ; x(., 2t+1)]
      cols 0-63  (A): rows 0-63 = wA(ky,0), rows 64-127 = wA(ky,1)
      cols 64-127(B): rows 0-63 = 0,        rows 64-127 = wB(ky,0)
  M2 @ x1[rc(h+ky, 2t+2)] : rhs = [x(., 2t+2) ; x(., 2t+3)]
      cols 0-63  (A): rows 0-63 = wA(ky,2), rows 64-127 = 0
      cols 64-127(B): rows 0-63 = wB(ky,1), rows 64-127 = wB(ky,2)

All stationaries are 128 columns (FWL fast weight load) and uniform shape,
which keeps LDWEIGHTS/MATMUL pipelined on the PE (~50ns/MM measured).
The zero quadrants are baked into the HBM weight image so each chunk is ONE
fully contiguous DMA: skipping them fragments the destination into 128-384B
runs (measured 75k-descriptor blowup), and matmul weight APs must be a
single contiguous free dim, so the +25% zero bytes are the cheaper trade.

x1 = [x_slab ; shift1(x_slab)]: only the unshifted top half crosses HBM;
the bottom half is built by the otherwise-idle vector engine (cross-
partition tensor_copy). Bias adds alternate ACT/DVE so neither engine
serializes PSUM release; output DMAs issue on the scalar HWDGE queue so
they never stall the sync queue streaming the weights.

Measured on TRN2: ~59-64us HW exec (baseline fp32 kernel: 175-205us).
Breakdown: ~17MB/core of DMA payload streams at queue saturation
(~40us) and dominates; tensor (384 matmuls, LDW-bound) ~21us hides
under it; ~7us fixed engine preamble + ~4us fill/drain.
"""

import sys

sys.path.insert(0, "/opt/trn_rl_repo")

import numpy as np
import ml_dtypes

BF16 = ml_dtypes.bfloat16

B, CIN, COUT, H, W = 64, 64, 64, 32, 32
K = 3
HOUT, WOUT = 32, 32
NCORES = 8
ROWS = HOUT // NCORES  # output rows per core
SLAB_R = ROWS + 2      # x rows needed per core (halo)
SLAB_C = W + 2         # padded width
RC = SLAB_R * SLAB_C   # flattened (row, col) length
NT = WOUT // 2         # column pairs per row
NQ = 2                 # weight chunks per output row
TG = NT // NQ          # pairs per weight chunk (chunk = (h, q))
XSPLIT = 78            # x1 first-piece size: covers the first half-chunk

_nc_cache = {}


def _build_bass():
    import concourse.bass as bass
    import concourse.tile as tile
    from concourse import bacc, mybir

    f32 = mybir.dt.float32
    bf16 = mybir.dt.bfloat16
    nc = bacc.Bacc(None, target_bir_lowering=False)

    x0_d = nc.dram_tensor("x0", (64, RC, B), bf16, kind="ExternalInput")
    wm_d = nc.dram_tensor(
        "wm", (ROWS, NQ, 128, TG, 3, 2, 128), bf16, kind="ExternalInput"
    )
    bias_d = nc.dram_tensor("bias", (128, ROWS, NT), f32, kind="ExternalInput")
    out_d = nc.dram_tensor("out", (ROWS, 128, NT, B), bf16, kind="ExternalOutput")

    with tile.TileContext(nc) as tc:
        with (
            tc.tile_pool(name="xpool", bufs=1) as xpool,
            tc.tile_pool(name="wpool", bufs=4) as wpool,
            tc.tile_pool(name="opool", bufs=2) as opool,
            tc.tile_pool(name="bpool", bufs=1) as bpool,
            tc.tile_pool(name="psum", bufs=8, space=bass.MemorySpace.PSUM) as psum,
        ):
            x1 = xpool.tile([128, RC, B], bf16, tag="x1")
            # Only the unshifted top half crosses HBM; the shift-by-one
            # bottom half is built by the (otherwise idle) vector engine.
            # First piece covers just pairs t=0,1 of row 0 so the first
            # matmul's dependency chain is ~1MB.
            nc.sync.dma_start(x1[0:64, 0:XSPLIT, :], x0_d[:, 0:XSPLIT, :])
            nc.vector.tensor_copy(
                x1[64:128, 0 : XSPLIT - 1, :], x1[0:64, 1:XSPLIT, :]
            )

            bi = bpool.tile([128, ROWS, NT], f32, tag="bias")
            nc.scalar.dma_start(bi[:], bias_d[:])

            for h in range(ROWS):
                ot = opool.tile([128, NT, B], bf16, tag="out", name="ot")
                for q in range(NQ):
                    wm = wpool.tile([128, TG, 3, 2, 128], bf16, tag="wm", name="wm")
                    if h == 0 and q == 0:
                        # split the very first chunk so compute starts
                        # sooner; the rest of x1 queues behind it. More
                        # aggressive splitting measurably regresses: each
                        # dma_start costs ~1-2.5us of HWDGE descriptor-gen
                        # on the sequencer, so keep DMA count minimal.
                        nc.sync.dma_start(
                            wm[:, 0 : TG // 2], wm_d[h, q, :, 0 : TG // 2]
                        )
                        nc.sync.dma_start(
                            wm[:, TG // 2 : TG], wm_d[h, q, :, TG // 2 : TG]
                        )
                        nc.sync.dma_start(
                            x1[0:64, XSPLIT:RC, :], x0_d[:, XSPLIT:RC, :]
                        )
                        nc.vector.tensor_copy(
                            x1[64:128, XSPLIT - 1 : RC - 1, :],
                            x1[0:64, XSPLIT:RC, :],
                        )
                    elif h == ROWS - 1 and q == NQ - 1:
                        # split the last chunk too: its matmuls otherwise
                        # all wait on the full 3.15MB transfer (one
                        # completion semaphore), serializing the tail
                        nc.sync.dma_start(
                            wm[:, 0 : TG // 2], wm_d[h, q, :, 0 : TG // 2]
                        )
                        nc.sync.dma_start(
                            wm[:, TG // 2 : TG], wm_d[h, q, :, TG // 2 : TG]
                        )
                    else:
                        nc.sync.dma_start(wm[:], wm_d[h, q])

                    for tt in range(TG):
                        t = q * TG + tt
                        ps = psum.tile([128, B], f32, tag="ps")
                        j = 0
                        for ky in range(3):
                            for m in range(2):
                                rc = (h + ky) * SLAB_C + 2 * t + 2 * m
                                nc.tensor.matmul(
                                    ps[:],
                                    wm[:, tt, ky, m, :],
                                    x1[:, rc, :],
                                    start=(j == 0),
                                    stop=(j == 5),
                                )
                                j += 1
                        if t % 2 == 0:
                            nc.scalar.activation(
                                ot[:, t, :],
                                ps[:],
                                mybir.ActivationFunctionType.Identity,
                                bias=bi[:, h, t : t + 1],
                            )
                        else:
                            nc.vector.tensor_scalar_add(
                                ot[:, t, :], ps[:], bi[:, h, t : t + 1]
                            )

                # issue on the scalar (ACT) HWDGE queue: its wait (the
                # bias adds) is already satisfied in FIFO order, so it
                # never stalls the sync queue streaming the weights
                # (gpsimd SWDGE was tried and is slower: ~2us fixed
                # cost per DMA).
                nc.scalar.dma_start(out_d[h], ot[:])

    nc.compile()
    return nc


def get_nc():
    if "nc" not in _nc_cache:
        _nc_cache["nc"] = _build_bass()
    return _nc_cache["nc"]


def pack_inputs(x, weight, bias):
    """Returns list of per-core in_maps (numpy, C-contiguous)."""
    x = np.asarray(x, dtype=np.float32)
    weight = np.asarray(weight, dtype=np.float32)
    bias = np.asarray(bias, dtype=np.float32)

    # padded x: [B, CIN, H+2, W+2]
    xp = np.zeros((B, CIN, H + 2, W + 2), dtype=np.float32)
    xp[:, :, 1:-1, 1:-1] = x

    # weight -> [h, w, ky, kx, cin, cout] in bf16
    wt_all = np.ascontiguousarray(
        np.transpose(weight, (2, 3, 4, 5, 0, 1))
    ).astype(BF16)

    in_maps = []
    for c in range(NCORES):
        h0 = c * ROWS
        # x slab rows h0-1 .. h0+ROWS (SLAB_R rows of padded x)
        slab = xp[:, :, h0 : h0 + SLAB_R, :]  # [B, CIN, SLAB_R, SLAB_C]
        x0 = np.transpose(slab, (1, 2, 3, 0)).reshape(CIN, RC, B).astype(BF16)

        wh = wt_all[h0 : h0 + ROWS]  # [ROWS, w, ky, kx, cin, cout]
        wA = wh[:, 0::2]  # [ROWS, NT, ky, kx, cin, cout]
        wB = wh[:, 1::2]

        def pk(a):  # [ROWS, NT, ky, cin, cout] -> [ROWS, cin, NT, ky, cout]
            return np.transpose(a, (0, 3, 1, 2, 4))

        # stationary image with zero quadrants baked in
        wm = np.zeros((ROWS, 128, NT, 3, 2, 128), dtype=BF16)
        wm[:, 0:64, :, :, 0, 0:64] = pk(wA[:, :, :, 0])
        wm[:, 64:128, :, :, 0, 0:64] = pk(wA[:, :, :, 1])
        wm[:, 64:128, :, :, 0, 64:128] = pk(wB[:, :, :, 0])
        wm[:, 0:64, :, :, 1, 0:64] = pk(wA[:, :, :, 2])
        wm[:, 0:64, :, :, 1, 64:128] = pk(wB[:, :, :, 1])
        wm[:, 64:128, :, :, 1, 64:128] = pk(wB[:, :, :, 2])
        # [ROWS, 128, NT, 3, 2, 128] -> [ROWS, NQ, 128, TG, 3, 2, 128]
        wm = wm.reshape(ROWS, 128, NQ, TG, 3, 2, 128).transpose(0, 2, 1, 3, 4, 5, 6)

        # bias image [128, ROWS, NT]: partition s*64+o -> (w=2t+s, cout=o)
        bh = bias[:, h0 : h0 + ROWS, :]  # [cout, ROWS, W]
        bimg = np.concatenate([bh[:, :, 0::2], bh[:, :, 1::2]], axis=0)

        in_maps.append(
            {
                "x0": np.ascontiguousarray(x0),
                "wm": np.ascontiguousarray(wm),
                "bias": np.ascontiguousarray(bimg),
            }
        )
    return in_maps


def unpack_outputs(results):
    """results: list of per-core out_maps with 'out' [ROWS, 128, NT, B] bf16."""
    full = np.concatenate([np.asarray(r["out"]) for r in results], axis=0)
    # [HOUT, 2, COUT, NT, B] -> [B, COUT, HOUT, NT, 2]
    o = full.reshape(HOUT, 2, COUT, NT, B)
    y = np.transpose(o, (4, 2, 0, 3, 1)).reshape(B, COUT, HOUT, WOUT)
    return np.ascontiguousarray(y.astype(np.float32))


def run(in_maps, **kwargs):
    from concourse import bass_utils

    nc = get_nc()
    return bass_utils.run_bass_kernel_spmd(
        nc, in_maps, core_ids=list(range(NCORES)), **kwargs
    )


def kernel(x, weight, bias):
    in_maps = pack_inputs(x, weight, bias)
    res = run(in_maps)
    return unpack_outputs(res.results)


if __name__ == "__main__":
    rng = np.random.default_rng(0)
    x = rng.standard_normal((B, CIN, H, W), dtype=np.float32)
    weight = rng.standard_normal((CIN, COUT, HOUT, WOUT, K, K), dtype=np.float32)
    bias = rng.standard_normal((COUT, HOUT, WOUT), dtype=np.float32)
    y = kernel(x, weight, bias)
    print("out", y.shape, y.dtype)


# revision 55
# speedup vs baseline: 1.0010x; 1.0010x over previous
"""LocallyConnected2d Trainium2 kernel (bf16 W-PAIR, contiguous weight image).

y[b,o,h,w] = sum_{i,ky,kx} x[b,i,h+ky-1,w+kx-1] * weight[i,o,h,w,ky,kx] + bias[o,h,w]

Shapes: x [64,64,32,32], weight [64,64,32,32,3,3], bias [64,32,32] -> y [64,64,32,32].

Strategy
--------
Spatial sharding over H_out: 8 cores x 4 output rows each (x slab with halo).
All compute in bf16 (fp32 matmuls are 4 cycles/row on TRN2 vs 1 for bf16;
rel-err budget 2e-2 >> bf16's ~0.3%), PSUM accumulation in fp32.

Output columns processed in pairs (A=2t, B=2t+1) sharing one PSUM tile
[128, 64] = [couts of A | couts of B] x batch. Per pair and kernel row ky,
TWO matmuls with K=128, M=128, N=64 cover all six (ky, kx) contributions:

  M1 @ x1[rc(h+ky, 2t)]   : rhs = [x(., 2t) ; x(., 2t+1)]
      cols 0-63  (A): rows 0-63 = wA(ky,0), rows 64-127 = wA(ky,1)
      cols 64-127(B): rows 0-63 = 0,        rows 64-127 = wB(ky,0)
  M2 @ x1[rc(h+ky, 2t+2)] : rhs = [x(., 2t+2) ; x(., 2t+3)]
      cols 0-63  (A): rows 0-63 = wA(ky,2), rows 64-127 = 0
      cols 64-127(B): rows 0-63 = wB(ky,1), rows 64-127 = wB(ky,2)

All stationaries are 128 columns (FWL fast weight load) and uniform shape,
which keeps LDWEIGHTS/MATMUL pipelined on the PE (~50ns/MM measured).
The zero quadrants are baked into the HBM weight image so each chunk is ONE
fully contiguous DMA: skipping them fragments the destination into 128-384B
runs (measured 75k-descriptor blowup), and matmul weight APs must be a
single contiguous free dim, so the +25% zero bytes are the cheaper trade.

x1 = [x_slab ; shift1(x_slab)]: only the unshifted top half crosses HBM;
the bottom half is built by the otherwise-idle vector engine (cross-
partition tensor_copy). Bias adds alternate ACT/DVE so neither engine
serializes PSUM release; output DMAs issue on the scalar HWDGE queue so
they never stall the sync queue streaming the weights.

Measured on TRN2: ~59-64us HW exec (baseline fp32 kernel: 175-205us).
Breakdown: ~17MB/core of DMA payload streams at queue saturation
(~40us) and dominates; tensor (384 matmuls, LDW-bound) ~21us hides
under it; ~7us fixed engine preamble + ~4us fill/drain.
"""

import sys

sys.path.insert(0, "/opt/trn_rl_repo")

import numpy as np
import ml_dtypes

BF16 = ml_dtypes.bfloat16

B, CIN, COUT, H, W = 64, 64, 64, 32, 32
K = 3
HOUT, WOUT = 32, 32
NCORES = 8
ROWS = HOUT // NCORES  # output rows per core
SLAB_R = ROWS + 2      # x rows needed per core (halo)
SLAB_C = W + 2         # padded width
RC = SLAB_R * SLAB_C   # flattened (row, col) length
NT = WOUT // 2         # column pairs per row
NQ = 2                 # weight chunks per output row
TG = NT // NQ          # pairs per weight chunk (chunk = (h, q))
XSPLIT = 78            # x1 first-piece size: covers the first half-chunk

_nc_cache = {}


def _build_bass():
    import concourse.bass as bass
    import concourse.tile as tile
    from concourse import bacc, mybir

    f32 = mybir.dt.float32
    bf16 = mybir.dt.bfloat16
    nc = bacc.Bacc(None, target_bir_lowering=False)

    x0_d = nc.dram_tensor("x0", (64, RC, B), bf16, kind="ExternalInput")
    wm_d = nc.dram_tensor(
        "wm", (ROWS, NQ, 128, TG, 3, 2, 128), bf16, kind="ExternalInput"
    )
    bias_d = nc.dram_tensor("bias", (128, ROWS, NT), f32, kind="ExternalInput")
    out_d = nc.dram_tensor("out", (ROWS, 128, NT, B), bf16, kind="ExternalOutput")

    with tile.TileContext(nc) as tc:
        with (
            tc.tile_pool(name="xpool", bufs=1) as xpool,
            tc.tile_pool(name="wpool", bufs=4) as wpool,
            tc.tile_pool(name="opool", bufs=2) as opool,
            tc.tile_pool(name="bpool", bufs=1) as bpool,
            tc.tile_pool(name="psum", bufs=8, space=bass.MemorySpace.PSUM) as psum,
        ):
            x1 = xpool.tile([128, RC, B], bf16, tag="x1")
            # Only the unshifted top half crosses HBM; the shift-by-one
            # bottom half is built by the (otherwise idle) vector engine.
            # First piece covers just pairs t=0,1 of row 0 so the first
            # matmul's dependency chain is ~1MB.
            nc.sync.dma_start(x1[0:64, 0:XSPLIT, :], x0_d[:, 0:XSPLIT, :])
            nc.vector.tensor_copy(
                x1[64:128, 0 : XSPLIT - 1, :], x1[0:64, 1:XSPLIT, :]
            )

            bi = bpool.tile([128, ROWS, NT], f32, tag="bias")
            nc.scalar.dma_start(bi[:], bias_d[:])

            for h in range(ROWS):
                ot = opool.tile([128, NT, B], bf16, tag="out", name="ot")
                for q in range(NQ):
                    wm = wpool.tile([128, TG, 3, 2, 128], bf16, tag="wm", name="wm")
                    if h == 0 and q == 0:
                        # split the very first chunk so compute starts
                        # sooner; the rest of x1 queues behind it. More
                        # aggressive splitting measurably regresses: each
                        # dma_start costs ~1-2.5us of HWDGE descriptor-gen
                        # on the sequencer, so keep DMA count minimal.
                        nc.sync.dma_start(
                            wm[:, 0 : TG // 2], wm_d[h, q, :, 0 : TG // 2]
                        )
                        nc.sync.dma_start(
                            wm[:, TG // 2 : TG], wm_d[h, q, :, TG // 2 : TG]
                        )
                        nc.sync.dma_start(
                            x1[0:64, XSPLIT:RC, :], x0_d[:, XSPLIT:RC, :]
                        )
                        nc.vector.tensor_copy(
                            x1[64:128, XSPLIT - 1 : RC - 1, :],
                            x1[0:64, XSPLIT:RC, :],
                        )
                    elif h == ROWS - 1 and q == NQ - 1:
                        # split the last chunk too: its matmuls otherwise
                        # all wait on the full 3.15MB transfer (one
                        # completion semaphore), serializing the tail
                        nc.sync.dma_start(
                            wm[:, 0 : TG // 2], wm_d[h, q, :, 0 : TG // 2]
                        )
                        nc.sync.dma_start(
                            wm[:, TG // 2 : TG], wm_d[h, q, :, TG // 2 : TG]
                        )
                    else:
                        nc.sync.dma_start(wm[:], wm_d[h, q])

                    for tt in range(TG):
                        t = q * TG + tt
                        ps = psum.tile([128, B], f32, tag="ps")
                        j = 0
                        for ky in range(3):
                            for m in range(2):
                                rc = (h + ky) * SLAB_C + 2 * t + 2 * m
                                nc.tensor.matmul(
                                    ps[:],
                                    wm[:, tt, ky, m, :],
                                    x1[:, rc, :],
                                    start=(j == 0),
                                    stop=(j == 5),
                                )
                                j += 1
                        if t % 2 == 0:
                            nc.scalar.activation(
                                ot[:, t, :],
                                ps[:],
                                mybir.ActivationFunctionType.Identity,
                                bias=bi[:, h, t : t + 1],
                            )
                        else:
                            nc.vector.tensor_scalar_add(
                                ot[:, t, :], ps[:], bi[:, h, t : t + 1]
                            )

                    # issue on the scalar (ACT) HWDGE queue: its wait (the
                    # bias adds) is already satisfied in FIFO order, so it
                    # never stalls the sync queue streaming the weights
                    # (gpsimd SWDGE was tried and is slower: ~2us fixed
                    # cost per DMA). Per-half-row slices shorten the tail;
                    # per-row slices measured the same within noise.
                    nc.scalar.dma_start(
                        out_d[h, :, q * TG : (q + 1) * TG, :],
                        ot[:, q * TG : (q + 1) * TG, :],
                    )

    nc.compile()
    return nc


def get_nc():
    if "nc" not in _nc_cache:
        _nc_cache["nc"] = _build_bass()
    return _nc_cache["nc"]


def pack_inputs(x, weight, bias):
    """Returns list of per-core in_maps (numpy, C-contiguous)."""
    x = np.asarray(x, dtype=np.float32)
    weight = np.asarray(weight, dtype=np.float32)
    bias = np.asarray(bias, dtype=np.float32)

    # padded x: [B, CIN, H+2, W+2]
    xp = np.zeros((B, CIN, H + 2, W + 2), dtype=np.float32)
    xp[:, :, 1:-1, 1:-1] = x

    # weight -> [h, w, ky, kx, cin, cout] in bf16
    wt_all = np.ascontiguousarray(
        np.transpose(weight, (2, 3, 4, 5, 0, 1))
    ).astype(BF16)

    in_maps = []
    for c in range(NCORES):
        h0 = c * ROWS
        # x slab rows h0-1 .. h0+ROWS (SLAB_R rows of padded x)
        slab = xp[:, :, h0 : h0 + SLAB_R, :]  # [B, CIN, SLAB_R, SLAB_C]
        x0 = np.transpose(slab, (1, 2, 3, 0)).reshape(CIN, RC, B).astype(BF16)

        wh = wt_all[h0 : h0 + ROWS]  # [ROWS, w, ky, kx, cin, cout]
        wA = wh[:, 0::2]  # [ROWS, NT, ky, kx, cin, cout]
        wB = wh[:, 1::2]

        def pk(a):  # [ROWS, NT, ky, cin, cout] -> [ROWS, cin, NT, ky, cout]
            return np.transpose(a, (0, 3, 1, 2, 4))

        # stationary image with zero quadrants baked in
        wm = np.zeros((ROWS, 128, NT, 3, 2, 128), dtype=BF16)
        wm[:, 0:64, :, :, 0, 0:64] = pk(wA[:, :, :, 0])
        wm[:, 64:128, :, :, 0, 0:64] = pk(wA[:, :, :, 1])
        wm[:, 64:128, :, :, 0, 64:128] = pk(wB[:, :, :, 0])
        wm[:, 0:64, :, :, 1, 0:64] = pk(wA[:, :, :, 2])
        wm[:, 0:64, :, :, 1, 64:128] = pk(wB[:, :, :, 1])
        wm[:, 64:128, :, :, 1, 64:128] = pk(wB[:, :, :, 2])
        # [ROWS, 128, NT, 3, 2, 128] -> [ROWS, NQ, 128, TG, 3, 2, 128]
        wm = wm.reshape(ROWS, 128, NQ, TG, 3, 2, 128).transpose(0, 2, 1, 3, 4, 5, 6)

        # bias image [128, ROWS, NT]: partition s*64+o -> (w=2t+s, cout=o)
        bh = bias[:, h0 : h0 + ROWS, :]  # [cout, ROWS, W]
        bimg = np.concatenate([bh[:, :, 0::2], bh[:, :, 1::2]], axis=0)

        in_maps.append(
            {
                "x0": np.ascontiguousarray(x0),
                "wm": np.ascontiguousarray(wm),
                "bias": np.ascontiguousarray(bimg),
            }
        )
    return in_maps


def unpack_outputs(results):
    """results: list of per-core out_maps with 'out' [ROWS, 128, NT, B] bf16."""
    full = np.concatenate([np.asarray(r["out"]) for r in results], axis=0)
    # [HOUT, 2, COUT, NT, B] -> [B, COUT, HOUT, NT, 2]
    o = full.reshape(HOUT, 2, COUT, NT, B)
    y = np.transpose(o, (4, 2, 0, 3, 1)).reshape(B, COUT, HOUT, WOUT)
    return np.ascontiguousarray(y.astype(np.float32))


def run(in_maps, **kwargs):
    from concourse import bass_utils

    nc = get_nc()
    return bass_utils.run_bass_kernel_spmd(
        nc, in_maps, core_ids=list(range(NCORES)), **kwargs
    )


def kernel(x, weight, bias):
    in_maps = pack_inputs(x, weight, bias)
    res = run(in_maps)
    return unpack_outputs(res.results)


if __name__ == "__main__":
    rng = np.random.default_rng(0)
    x = rng.standard_normal((B, CIN, H, W), dtype=np.float32)
    weight = rng.standard_normal((CIN, COUT, HOUT, WOUT, K, K), dtype=np.float32)
    bias = rng.standard_normal((COUT, HOUT, WOUT), dtype=np.float32)
    y = kernel(x, weight, bias)
    print("out", y.shape, y.dtype)
